# revision 21
# baseline (speedup 1.0000x reference)
"""Trainium2 Bass kernel for a dense multi-head attention layer.

Problem (hardcoded shapes):
    hidden_states [2, 2048, 2048] fp32, attention_mask [2,1,1,2048] int32 (all ones),
    Wq/Wk/Wv/Wo [2048, 2048] fp32, biases [2048] fp32 (zeros in practice).
    out = MHA(hidden) with H=16 heads, head_dim=128.

Sharding: 8 cores = 2 batches x 4 head-groups (4 heads per core, tensor
parallel over heads). Each core computes q/k/v projections for its 4 heads,
attention, and a partial output projection; the host sums the 4 partials per
batch.

All matmuls run in fp32r (E8M11, full PE speed); inputs are pre-rounded on
the host, on-chip matmul operands are produced directly in fp32r by the
ACT/DVE ops that already exist in the pipeline. PSUM accumulation is fp32.

Layout trick: everything is arranged so no on-device transpose is ever
needed. The host supplies hidden^T and pre-transposed weights; scores are
computed keys-major (sT = kT^T @ qT) so that the PV matmul consumes exp(sT)
directly and produces attn^T, which is exactly the lhsT layout the output
projection wants. Softmax denominators come from a ones-vector matmul over
the partition axis; normalization is folded into the PSUM->SBUF eviction.
"""
import os
import sys

if "/opt/trn_rl_repo" not in sys.path:
    sys.path.insert(0, "/opt/trn_rl_repo")

# If a previous run crashed the NEFF execution, a fresh NRT open with this
# flag recovers the cores instead of failing with EXEC_UNIT_UNRECOVERABLE.
os.environ.setdefault("NEURON_RT_RESET_CORES", "1")

import numpy as np

B, S, D, H, HD = 2, 2048, 2048, 16, 128
NCORES = 8
GROUPS = 4            # head-groups == cores per batch
GH = H // GROUPS      # heads per core = 4
GD = GH * HD          # 512 projection cols per core
ST = 512              # s/q/o tile width (fp32 matmul free-dim max)
NSB = S // 128        # 16 s-blocks
NEB = D // 128        # 16 e-blocks (contraction)
NST = S // ST         # 4 s-tiles
SCALE = 1.0 / float(np.sqrt(HD))

_RUNNER = None


def _round_fp32r(x: np.ndarray) -> np.ndarray:
    """Round fp32 to fp32r (E8M11): round-to-nearest-even to 11 mantissa bits."""
    b = np.ascontiguousarray(x, dtype=np.float32).view(np.uint32)
    lsb = (b >> np.uint32(12)) & np.uint32(1)
    r = (b + np.uint32(0x7FF) + lsb) & np.uint32(0xFFFFF000)
    return r.view(np.float32)


def _build_nc():
    import os
    import concourse.tile as tile
    import concourse.bass_isa as bass_isa
    from concourse import bacc, mybir

    phases = os.environ.get("K_PHASES", "123")

    f32 = mybir.dt.float32
    f32r = mybir.dt.float32r
    Exp = mybir.ActivationFunctionType.Exp

    nc = bacc.Bacc("TRN2", target_bir_lowering=False, debug=False,
                   num_devices=NCORES)

    hT = nc.dram_tensor("hT", [D, S], f32r, kind="ExternalInput")
    wqT = nc.dram_tensor("wqT", [D, GD], f32r, kind="ExternalInput")
    wkT = nc.dram_tensor("wkT", [D, GD], f32r, kind="ExternalInput")
    wvT = nc.dram_tensor("wvT", [D, GD], f32r, kind="ExternalInput")
    woT = nc.dram_tensor("woT", [GD, D], f32r, kind="ExternalInput")
    out = nc.dram_tensor("out", [S, D], f32, kind="ExternalOutput")

    with tile.TileContext(nc) as tc:
        with tc.tile_pool(name="dram", bufs=1, space="DRAM") as dram, \
             tc.tile_pool(name="persist", bufs=1) as persist:
            qT_s = dram.tile([GH, 128, S], f32r)
            kT_s = dram.tile([GH, 128, S], f32r)
            v_s = dram.tile([S, GD], f32r)


            # ---------------- phase 1: q/k/v projections ----------------
            with tc.tile_pool(name="w1", bufs=1) as w1, \
                 tc.tile_pool(name="hslab", bufs=2) as hpool, \
                 tc.tile_pool(name="ev1", bufs=4) as ev1, \
                 tc.tile_pool(name="ps1", bufs=8, space="PSUM") as ps1:
                wq_sb = wqk.tile([128, NEB, GD], f32r)
                wk_sb = wqk.tile([128, NEB, GD], f32r)
                wv_sb = wv1.tile([128, NEB, GD], f32r)
                wqT_r = wqT.rearrange("(n p) d -> p n d", p=128)
                wkT_r = wkT.rearrange("(n p) d -> p n d", p=128)
                wvT_r = wvT.rearrange("(n p) d -> p n d", p=128)
                hT_r = hT.rearrange("(n p) s -> p n s", p=128)

                first_slab = hpool.tile([128, NEB, ST], f32r, tag="hslab")
                for eb in range(NEB):
                    # interleave so eb=0 pieces of the q/k path arrive first
                    nc.sync.dma_start(out=first_slab[:, eb, :],
                                      in_=hT_r[:, eb, 0:ST])
                    nc.sync.dma_start(out=wq_sb[:, eb, :],
                                      in_=wqT_r[:, eb, :])
                    nc.sync.dma_start(out=wk_sb[:, eb, :],
                                      in_=wkT_r[:, eb, :])
                for eb in range(NEB):
                    nc.sync.dma_start(out=wv_sb[:, eb, :],
                                      in_=wvT_r[:, eb, :])

                for st in range(NST):
                    if st == 0:
                        h_sb = first_slab
                    else:
                        h_sb = hpool.tile([128, NEB, ST], f32r, tag="hslab")
                        for eb in range(NEB):
                            nc.sync.dma_start(
                                out=h_sb[:, eb, :],
                                in_=hT_r[:, eb, st * ST:(st + 1) * ST])
                    if st == 0:
                        # eb-outer over 8 live accumulators: consume input
                        # chunks in arrival order so the PE tracks the DMA
                        # stream instead of stalling per accumulation.
                        pss = {}
                        for h in range(GH):
                            for t in range(2):
                                pss[(h, t)] = ps1.tile([128, ST], f32,
                                                       tag="ps1",
                                                       name=f"psqk{h}{t}")
                        for eb in range(NEB):
                            for h in range(GH):
                                for t, w_sb in ((0, wq_sb), (1, wk_sb)):
                                    nc.tensor.matmul(
                                        pss[(h, t)],
                                        w_sb[:, eb, h * HD:(h + 1) * HD],
                                        h_sb[:, eb, :],
                                        start=(eb == 0), stop=(eb == NEB - 1))
                        for h in range(GH):
                            for t, dst in ((0, qT_s), (1, kT_s)):
                                ev = ev1.tile([128, ST], f32r, tag="ev1")
                                nc.scalar.copy(ev, pss[(h, t)])
                                nc.scalar.dma_start(
                                    out=dst[h, :, st * ST:(st + 1) * ST],
                                    in_=ev)
                    else:
                      for h in range(GH):
                        for w_sb, dst in ((wq_sb, qT_s), (wk_sb, kT_s)):
                            ps = ps1.tile([128, ST], f32, tag="ps1")
                            for eb in range(NEB):
                                nc.tensor.matmul(
                                    ps, w_sb[:, eb, h * HD:(h + 1) * HD],
                                    h_sb[:, eb, :],
                                    start=(eb == 0), stop=(eb == NEB - 1))
                            ev = ev1.tile([128, ST], f32r, tag="ev1")
                            nc.scalar.copy(ev, ps)
                            nc.scalar.dma_start(
                                out=dst[h, :, st * ST:(st + 1) * ST], in_=ev)
                    for j in range(ST // 128):
                        ps = ps1.tile([128, GD], f32, tag="ps1")
                        for eb in range(NEB):
                            nc.tensor.matmul(
                                ps, h_sb[:, eb, j * 128:(j + 1) * 128],
                                wv_sb[:, eb, :],
                                start=(eb == 0), stop=(eb == NEB - 1))
                        ev = ev1.tile([128, GD], f32r, tag="ev1")
                        nc.scalar.copy(ev, ps)
                        row = st * ST + j * 128
                        nc.scalar.dma_start(out=v_s[row:row + 128, :], in_=ev)

            # ------- phase 2+3: attention fused with output projection -------
            # qt outer / heads inner: the output projection for query tile qt
            # runs as soon as all heads finished that tile, filling the PE
            # while the (ACT-bound) exp stream of the next tile runs.
            if "2" in phases:
              with tc.tile_pool(name="kqv", bufs=1) as kqv, \
                 tc.tile_pool(name="qsl", bufs=4) as qsl, \
                 tc.tile_pool(name="expp", bufs=3) as expp, \
                 tc.tile_pool(name="sm", bufs=2) as sm, \
                 tc.tile_pool(name="attn2", bufs=2) as attn2, \
                 tc.tile_pool(name="wo", bufs=1) as wo_p, \
                 tc.tile_pool(name="ev3", bufs=2) as ev3, \
                 tc.tile_pool(name="ps_s", bufs=2, space="PSUM") as ps_s, \
                 tc.tile_pool(name="acc", bufs=4, space="PSUM") as acc:
                kts, vts, qs0 = [], [], []
                for h in range(GH):
                    kt = kqv.tile([128, NSB, 128], f32r, name=f"kt{h}")
                    vt = kqv.tile([128, NSB, 128], f32r, name=f"vt{h}")
                    kT_s_r = kT_s[h].rearrange("p (n c) -> p n c", c=128)
                    v_s_r = v_s[:, h * HD:(h + 1) * HD].rearrange(
                        "(n p) d -> p n d", p=128)
                    for c4 in range(4):
                        blk = slice(c4 * 4, (c4 + 1) * 4)
                        nc.sync.dma_start(out=kt[:, blk, :],
                                          in_=kT_s_r[:, blk, :])
                    qs = qsl.tile([128, ST], f32r, tag="qs", name=f"qs0{h}")
                    nc.sync.dma_start(out=qs, in_=qT_s[h][:, 0:ST])
                    for c4 in range(4):
                        blk = slice(c4 * 4, (c4 + 1) * 4)
                        nc.sync.dma_start(out=vt[:, blk, :],
                                          in_=v_s_r[:, blk, :])
                    kts.append(kt)
                    vts.append(vt)
                    qs0.append(qs)
                wo_sb = []
                woT_r = woT.rearrange("(n p) o -> p n o", p=128)

                def ph3_block(prev, qt_prev, j):
                    # output projection for query tile qt_prev, s-block j
                    sb = qt_prev * (ST // 128) + j
                    ov = ev3.tile([128, D], f32, tag="ov", name=f"ov{sb}")
                    for ot in range(NST):
                        po = acc.tile([128, ST], f32, tag="acc",
                                      name=f"po{sb}{ot}")
                        for cb in range(GH):
                            nc.tensor.matmul(
                                po,
                                prev[cb][:, j * 128:(j + 1) * 128],
                                wo_sb[cb][:, ot * ST:(ot + 1) * ST],
                                start=(cb == 0), stop=(cb == GH - 1))
                        nc.scalar.copy(ov[:, ot * ST:(ot + 1) * ST], po)
                    nc.scalar.dma_start(
                        out=out[sb * 128:(sb + 1) * 128, :], in_=ov)

                prev_attn = None
                for qt in range(NST):
                    attn_t = []
                    for h in range(GH):
                        if qt == 0:
                            qs = qs0[h]
                        else:
                            qs = qsl.tile([128, ST], f32r, tag="qs")
                            nc.sync.dma_start(
                                out=qs,
                                in_=qT_s[h][:, qt * ST:(qt + 1) * ST])
                        expt_halves = [
                            expp.tile([128, NSB // 2, ST], f32r, tag="expt",
                                      name=f"expt{h}{half}")
                            for half in range(2)]
                        es = sm.tile([128, ST], f32, tag="es")
                        pa = acc.tile([128, ST], f32, tag="acc")
                        for kb2 in range(NSB // 2):
                            expt = expt_halves[kb2 // 4]
                            kbo = (kb2 % 4) * 2
                            ps = ps_s.tile([128, 2, ST], f32, tag="ps")
                            for half in range(2):
                                kb = kb2 * 2 + half
                                nc.tensor.matmul(ps[:, half, :],
                                                 kts[h][:, kb, :], qs,
                                                 start=True, stop=True)
                            pair = expt[:, kbo:kbo + 2, :]
                            nc.scalar.activation(pair, ps, Exp, scale=SCALE)
                            if kb2 == 0:
                                nc.vector.tensor_copy(
                                    es, expt[:, kbo, :].bitcast(f32))
                            else:
                                nc.vector.tensor_add(
                                    es, es, expt[:, kbo, :].bitcast(f32))
                            nc.vector.tensor_add(
                                es, es, expt[:, kbo + 1, :].bitcast(f32))
                            for half in range(2):
                                kb = kb2 * 2 + half
                                nc.tensor.matmul(pa, vts[h][:, kb, :],
                                                 expt[:, kbo + half, :],
                                                 start=(kb == 0),
                                                 stop=(kb == NSB - 1))
                        bcsum = sm.tile([128, ST], f32, tag="bcsum")
                        nc.gpsimd.partition_all_reduce(
                            bcsum, es, 128, bass_isa.ReduceOp.add)
                        brc = sm.tile([128, ST], f32, tag="brc")
                        nc.vector.reciprocal(brc, bcsum)
                        at = attn2.tile([128, ST], f32r, name=f"at{h}",
                                        tag=f"at{h}")
                        nc.vector.tensor_mul(at, pa, brc)
                        attn_t.append(at)
                        if prev_attn is not None:
                            ph3_block(prev_attn, qt - 1, h)
                    if qt == 0:
                        for cb in range(GH):
                            wt = wo_p.tile([128, D], f32r, name=f"wo{cb}")
                            nc.sync.dma_start(out=wt, in_=woT_r[:, cb, :])
                            wo_sb.append(wt)
                    prev_attn = attn_t
                for j in range(ST // 128):
                    ph3_block(prev_attn, NST - 1, j)
            else:
                with tc.tile_pool(name="zf", bufs=1) as zf:
                    z = zf.tile([128, D], f32)
                    nc.vector.memset(z, 0.0)
                    for sb in range(NSB):
                        nc.scalar.dma_start(out=out[sb * 128:(sb + 1) * 128, :],
                                            in_=z)

    nc.compile()
    return nc


def _get_runner():
    global _RUNNER
    if _RUNNER is None:
        _RUNNER = _build_nc()
    return _RUNNER


def _prepare_in_maps(hidden_states, Wq, Wk, Wv, Wo):
    hidden = np.asarray(hidden_states, dtype=np.float32)
    hT = [_round_fp32r(np.ascontiguousarray(hidden[b].T)) for b in range(B)]
    wq = np.asarray(Wq, dtype=np.float32)
    wk = np.asarray(Wk, dtype=np.float32)
    wv = np.asarray(Wv, dtype=np.float32)
    wo = np.asarray(Wo, dtype=np.float32)
    in_maps = []
    for core in range(NCORES):
        b, g = divmod(core, GROUPS)
        rows = slice(g * GD, (g + 1) * GD)
        in_maps.append({
            "hT": hT[b],
            "wqT": _round_fp32r(np.ascontiguousarray(wq[rows, :].T)),
            "wkT": _round_fp32r(np.ascontiguousarray(wk[rows, :].T)),
            "wvT": _round_fp32r(np.ascontiguousarray(wv[rows, :].T)),
            "woT": _round_fp32r(np.ascontiguousarray(wo[:, rows].T)),
        })
    return in_maps


def _run_device(in_maps, trace=False):
    from concourse.bass_utils import run_bass_kernel_spmd
    nc = _get_runner()
    try:
        return run_bass_kernel_spmd(nc, in_maps, core_ids=list(range(NCORES)),
                                    trace=trace)
    except Exception:
        # Transient device failures (rare) are recoverable by reopening the
        # backend with NEURON_RT_RESET_CORES=1. Retry once.
        try:
            import jax
            jax.clear_caches()
            try:
                jax.extend.backend.clear_backends()
            except Exception:
                jax._src.api.clear_backends()
        except Exception:
            pass
        return run_bass_kernel_spmd(nc, in_maps, core_ids=list(range(NCORES)),
                                    trace=trace)


def _numpy_reference(hidden_states, attention_mask, Wq, bq, Wk, bk, Wv, bv,
                     Wo, bo):
    """Exact fallback for inputs the fast path does not handle."""
    h = np.asarray(hidden_states, dtype=np.float32)
    mask = np.asarray(attention_mask)
    q = h @ np.asarray(Wq, np.float32).T + np.asarray(bq, np.float32)
    k = h @ np.asarray(Wk, np.float32).T + np.asarray(bk, np.float32)
    v = h @ np.asarray(Wv, np.float32).T + np.asarray(bv, np.float32)
    q = q.reshape(B, S, H, HD).transpose(0, 2, 1, 3)
    k = k.reshape(B, S, H, HD).transpose(0, 2, 1, 3)
    v = v.reshape(B, S, H, HD).transpose(0, 2, 1, 3)
    scores = (q @ k.transpose(0, 1, 3, 2)).astype(np.float32) * SCALE
    scores = np.where(mask == 0, np.float32(-1e9), scores)
    scores -= scores.max(axis=-1, keepdims=True)
    probs = np.exp(scores, dtype=np.float32)
    probs /= probs.sum(axis=-1, keepdims=True)
    attn = probs @ v
    attn = attn.transpose(0, 2, 1, 3).reshape(B, S, D)
    out = attn @ np.asarray(Wo, np.float32).T + np.asarray(bo, np.float32)
    return out.astype(np.float32)


def kernel(hidden_states, attention_mask, Wq, bq, Wk, bk, Wv, bv, Wo, bo):
    mask = np.asarray(attention_mask)
    bq_np = np.asarray(bq, dtype=np.float32)
    if (mask == 0).any() or np.any(bq_np):
        # general (never hit with the reference setup_inputs): bq shifts
        # scores per-key and a masked key changes the softmax support —
        # neither is representable in the fast path's fused layout.
        return _numpy_reference(hidden_states, attention_mask, Wq, bq, Wk,
                                bk, Wv, bv, Wo, bo)

    in_maps = _prepare_in_maps(hidden_states, Wq, Wk, Wv, Wo)
    res = _run_device(in_maps)

    # bk only adds a per-query constant to scores (softmax-invariant).
    # bv passes through the probs (rows sum to 1): out += bv @ Wo.T. bo adds.
    extra = (np.asarray(bv, np.float64) @ np.asarray(Wo, np.float64).T
             + np.asarray(bo, np.float64))
    out = np.empty((B, S, D), dtype=np.float32)
    for b in range(B):
        acc = np.zeros((S, D), dtype=np.float64)
        for g in range(GROUPS):
            acc += res.results[b * GROUPS + g]["out"]
        out[b] = (acc + extra).astype(np.float32)
    return out
